# revision 11
# baseline (speedup 1.0000x reference)
"""Trainium2 Bass kernel for nn_DSRB_19447611916345 (dense_cnn).

Math (per batch image, C=256, H=W=128):
    S    = 0.25*(conv1x1_s1(x)+...+conv1x1_s4(x))  four (+-2,+-2)-shifted 1x1 convs
    res  = tanh(0.5*(x - S))            (= 2*sigmoid(x-S)-1)
    h    = relu(x * res)
    gate = AGCA(mean_{H,W}(h))          tiny channel-attention MLP
    out  = h * gate

Sharding: data-parallel over batch B=8 across 8 cores, weights replicated.

Device kernel (per core, one image):
  - conv matmuls in fp8e4 DoubleRow perf mode: each matmul contracts
    K=256 (both 128-channel halves paired per partition) at 0.5 cyc/row;
    weights carry a -0.25*SWC factor (sign folded so PSUM = 2^13*(x-S)).
  - a 5th bf16 matmul per tile adds +8192*x via an identity lhsT, so the
    (x - S) subtract happens inside PSUM accumulation (PE, not DVE).
  - ACT reads PSUM directly: res = tanh(2^-14*psum - 0.5*bsum) -> bf16
  - DVE: hp = x*res (bf16), then h = relu(hp) with accum_out partial sums
  - h streams to HBM as bf16 during the main loop; no phase-2 tail.
  - AGCA gate + broadcast multiply run on host in f64/f32 (tiny + exact).
  - schedule: 6-tile deep prefetch, stores lagged one tile behind compute
    (avoids SP-sequencer head-of-line blocking), trailing stores split into
    4-row chunks; DMA runs gap-free at its transfer floor.
Host prep: fp8/bf16 casts, padded fp8 copy of x, weight transpose+scale.
"""

import numpy as np
import ml_dtypes

import concourse.bacc as bacc
import concourse.mybir as mybir
import concourse.tile as tile

f32 = mybir.dt.float32
bf16 = mybir.dt.bfloat16
f8 = mybir.dt.float8e4
Alu = mybir.AluOpType
Act = mybir.ActivationFunctionType
DR = mybir.MatmulPerfMode.DoubleRow

B = 8
C = 256
H = 128
W = 128
P = 128            # SBUF partitions
KH = C // P        # 2 input-channel halves (DoubleRow pair dim)
MH = C // P        # 2 output-channel halves
RB = 4             # output rows per PSUM tile
NBLK = H // RB     # 32
NT = RB * W        # 512 = PSUM bank
PADW = W + 4       # 132
PADH = H + 4       # 132
NXQ = PADH // 8 + 1   # 17 fp8 row-tiles (last has 4 rows)
SHIFTS = [(0, 0), (4, 0), (0, 4), (4, 4)]
STORE_LAG = 2      # store tile ii at block 2*(ii+STORE_LAG)+1
SX = 32.0          # fp8 scale on x
SWC = 2048.0       # weight scale; x unscaled in fp8 (exponent folding)
PSC = 2048.0       # PSUM carries 2^11*(x-S)

NP8 = ml_dtypes.float8_e4m3
NBF = ml_dtypes.bfloat16

_STATE = {}


def _build():
    nc = bacc.Bacc(name="dsrb2")
    xb_d = nc.dram_tensor("xb", [P, MH, PADH, PADW], bf16, kind="ExternalInput")
    wl_d = nc.dram_tensor("wl", [P, len(SHIFTS), MH, KH, P], f8,
                          kind="ExternalInput")
    idn_d = nc.dram_tensor("idn", [P, P], bf16, kind="ExternalInput")
    bneg_d = nc.dram_tensor("bneg", [P, MH], f32, kind="ExternalInput")
    hq_d = nc.dram_tensor("hq", [P, MH, H, PADW], bf16, kind="ExternalOutput")
    part_d = nc.dram_tensor("part", [P, MH, NBLK], f32, kind="ExternalOutput")

    with tile.TileContext(nc) as tc:
        with (
            tc.tile_pool(name="const", bufs=1) as constp,
            tc.tile_pool(name="xq", bufs=8) as xqp,
            tc.tile_pool(name="xb", bufs=8) as xbp,
            tc.tile_pool(name="hst", bufs=5) as hstp,
            tc.tile_pool(name="mm", bufs=4) as mmp,
            tc.tile_pool(name="ps", bufs=4, space="PSUM") as psp,
        ):
            xqts, xbts, hsts = {}, {}, {}

            ncast = [0]

            def load_xb(gg):
                """Load an 8-padded-row bf16 tile; cast it to the fp8 conv
                tile on Pool (2 of 3) or DVE (1 of 3) - both have slack."""
                rows = min(8, PADH - 8 * gg)
                t = xbp.tile([P, MH, rows, PADW], bf16, tag="xb")
                nc.sync.dma_start(out=t, in_=xb_d[:, :, 8 * gg : 8 * gg + rows, :])
                xbts[gg] = t
                q = xqp.tile([P, KH, rows, PADW], f8, tag="xq")
                ceng = nc.gpsimd if ncast[0] % 3 != 2 else nc.vector
                ncast[0] += 1
                ceng.tensor_copy(out=q, in_=t)
                xqts[gg] = q

            load_xb(0)
            wt = constp.tile([P, len(SHIFTS), MH, KH, P], f8)
            nc.sync.dma_start(out=wt, in_=wl_d[:])
            idn = constp.tile([P, P], bf16)
            nc.sync.dma_start(out=idn, in_=idn_d[:, :])
            bneg = constp.tile([P, MH], f32)
            nc.sync.dma_start(out=bneg, in_=bneg_d[:, :])
            partials = constp.tile([P, MH, NBLK], f32)
            for gg in range(1, 5):
                load_xb(gg)
            for _ in range(4):
                tz = tpp.tile([P, RB, PADW], bf16, tag="t")
                nc.scalar.memzero(tz)

            for j in range(NBLK):
                ii, ro = divmod(j, 2)
                ro *= RB
                if j % 2 == 0:
                    hst_t = hstp.tile([P, MH, 8, PADW], bf16, tag="hst")
                    hsts[ii] = hst_t
                    if ii + 4 < NXQ:
                        load_xb(ii + 4)
                for mh in range(MH):
                    ps = psp.tile([P, NT], f32)
                    for s, (dr, dw) in enumerate(SHIFTS):
                        gg, off = divmod(RB * j + dr, 8)
                        rhs = xqts[gg][:, :, off : off + RB, dw : dw + W]
                        nc.tensor.matmul(
                            ps, wt[:, s, mh, :, :], rhs,
                            start=(s == 0), stop=False, perf_mode=DR,
                        )
                    g0, off0 = divmod(RB * j + 2, 8)
                    n0 = min(RB, 8 - off0)
                    nc.tensor.matmul(
                        ps[:, 0 : n0 * W], idn,
                        xbts[g0][:, mh, off0 : off0 + n0, 2 : 2 + W],
                        start=False, stop=(n0 == RB),
                    )
                    if n0 < RB:
                        nc.tensor.matmul(
                            ps[:, n0 * W : NT], idn,
                            xbts[g0 + 1][:, mh, 0 : RB - n0, 2 : 2 + W],
                            start=False, stop=True,
                        )
                    t_t = tpp.tile([P, RB, PADW], bf16, tag="t")
                    nc.scalar.activation(
                        out=t_t[:, :, 2 : 2 + W], in_=ps, func=Act.Tanh,
                        bias=bneg[:, mh : mh + 1], scale=0.5 / PSC,
                    )
                    ttw = t_t.rearrange("p a b -> p (a b)")
                    hp_t = mmp.tile([P, RB * PADW], bf16, tag="hp")
                    nc.vector.tensor_tensor(
                        out=hp_t[:, 0 : n0 * PADW],
                        in0=xbts[g0][:, mh, off0 : off0 + n0, :],
                        in1=ttw[:, 0 : n0 * PADW], op=Alu.mult,
                    )
                    if n0 < RB:
                        nc.vector.tensor_tensor(
                            out=hp_t[:, n0 * PADW :],
                            in0=xbts[g0 + 1][:, mh, 0 : RB - n0, :],
                            in1=ttw[:, n0 * PADW :], op=Alu.mult,
                        )
                    hrow = hsts[ii][:, mh, ro : ro + RB, :]
                    nc.vector.tensor_scalar(
                        out=hrow.rearrange("p a b -> p (a b)"), in0=hp_t,
                        scalar1=0.0, scalar2=0.0,
                        op0=Alu.max, op1=Alu.add,
                        accum_out=partials[:, mh, j : j + 1],
                    )
                if j % 2 == 1:
                    si = ii - STORE_LAG
                    if si >= 0:
                        for half in range(2):
                            nc.sync.dma_start(
                                out=hq_d[:, :, 8 * si + 4 * half : 8 * si + 4 * half + 4, :],
                                in_=hsts[si][:, :, 4 * half : 4 * half + 4, :],
                            )
                        hsts.pop(si, None)
            for si in range(H // 8 - STORE_LAG, H // 8):
                for half in range(2):
                    nc.sync.dma_start(
                        out=hq_d[:, :, 8 * si + 4 * half : 8 * si + 4 * half + 4, :],
                        in_=hsts[si][:, :, 4 * half : 4 * half + 4, :],
                    )
                hsts.pop(si, None)
            nc.sync.dma_start(out=part_d[:], in_=partials)

    nc.finalize()
    return nc


def _prep_core_inputs(x_img, shared):
    """x_img: [C, H, W] f32 for one batch image."""
    x4 = x_img.reshape(KH, P, H, W).transpose(1, 0, 2, 3)  # [P, KH, H, W]
    xb = np.zeros((P, MH, PADH, PADW), NBF)
    xb[:, :, 2 : H + 2, 2 : W + 2] = x4.astype(NBF)
    return {"xb": xb, **shared}


def _prep_shared(inputs):
    ws = np.stack([np.asarray(inputs[f"w{i}"]) for i in (1, 2, 3, 4)]).astype(
        np.float64
    )
    # wl[p, s, mh, kh, m] = -0.25 * SWC * w_s[mh*P+m, kh*P+p]
    wl = (-0.25 * SWC * ws).reshape(
        len(SHIFTS), MH, P, KH, P
    ).transpose(4, 0, 1, 3, 2)
    wl = np.ascontiguousarray(wl).astype(NP8)
    idn = (PSC * np.eye(P)).astype(NBF)
    bsum = 0.25 * sum(np.asarray(inputs[f"b{i}"], np.float64) for i in (1, 2, 3, 4))
    bneg = np.ascontiguousarray((-0.5 * bsum).reshape(MH, P).T).astype(np.float32)
    return {"wl": wl, "idn": idn, "bneg": bneg}


def _get_runner(nc):
    """Cached shard_map-jitted executor mirroring bass2jax.run_bass_via_pjrt's
    multi-core path, so repeat kernel() calls don't re-trace/re-jit."""
    import jax
    import concourse.mybir as mb
    from concourse import bass2jax
    from jax.sharding import Mesh, PartitionSpec
    from jax.experimental.shard_map import shard_map

    bass2jax.install_neuronx_cc_hook()
    partition_name = (
        nc.partition_id_tensor.name if nc.partition_id_tensor else None
    )
    in_names, out_names, out_avals, zero_shapes = [], [], [], []
    for alloc in nc.m.functions[0].allocations:
        if not isinstance(alloc, mb.MemoryLocationSet):
            continue
        name = alloc.memorylocations[0].name
        if alloc.kind == "ExternalInput":
            if name != partition_name:
                in_names.append(name)
        elif alloc.kind == "ExternalOutput":
            out_names.append(name)
            shape = tuple(alloc.tensor_shape)
            dtype = mb.dt.np(alloc.dtype)
            out_avals.append(jax.core.ShapedArray(shape, dtype))
            zero_shapes.append((shape, dtype))
    n_params = len(in_names)
    n_outs = len(out_avals)
    all_in_names = list(in_names) + list(out_names)
    if partition_name is not None:
        all_in_names.append(partition_name)
    donate = tuple(range(n_params, n_params + n_outs))

    def _body(*args):
        operands = list(args)
        if partition_name is not None:
            operands.append(bass2jax.partition_id_tensor())
        outs = bass2jax._bass_exec_p.bind(
            *operands,
            out_avals=tuple(out_avals),
            in_names=tuple(all_in_names),
            out_names=tuple(out_names),
            lowering_input_output_aliases=(),
            sim_require_finite=True,
            sim_require_nnan=True,
            nc=nc,
        )
        return tuple(outs)

    devices = jax.devices()[:B]
    mesh = Mesh(np.asarray(devices), ("core",))
    in_specs = (PartitionSpec("core"),) * (n_params + n_outs)
    out_specs = (PartitionSpec("core"),) * n_outs
    sharded = jax.jit(
        shard_map(_body, mesh=mesh, in_specs=in_specs, out_specs=out_specs,
                  check_rep=False),
        donate_argnums=donate,
        keep_unused=True,
    )

    def run(in_maps):
        concat_in = [
            np.concatenate([np.asarray(in_maps[c][nm]) for c in range(B)], axis=0)
            for nm in in_names
        ]
        concat_zeros = [
            np.zeros((B * s[0], *s[1:]), d) for s, d in zero_shapes
        ]
        out_arrs = sharded(*concat_in, *concat_zeros)
        return [
            {
                nm: np.asarray(out_arrs[i]).reshape(B, *out_avals[i].shape)[c]
                for i, nm in enumerate(out_names)
            }
            for c in range(B)
        ]

    return run


def _bf16_to_f32(a_u16):
    return (a_u16.astype(np.uint32) << 16).view(np.float32)


def kernel(**inputs):
    if "nc" not in _STATE:
        _STATE["nc"] = _build()
    nc = _STATE["nc"]
    x = np.asarray(inputs["x"], np.float32)
    shared = _prep_shared(inputs)
    in_maps = [_prep_core_inputs(x[b], shared) for b in range(B)]
    if "runner" not in _STATE:
        _STATE["runner"] = _get_runner(nc)
    results = _STATE["runner"](in_maps)

    # ---- host: AGCA gate (f64, tiny) + broadcast gate multiply ----
    aw1 = np.asarray(inputs["agca_w1"], np.float64)
    w2v = float(np.asarray(inputs["agca_w2"])[0])
    w3v = float(np.asarray(inputs["agca_w3"])[0])
    a2 = np.asarray(inputs["agca_A2"], np.float64)
    aw4 = np.asarray(inputs["agca_w4"], np.float64)

    out = np.empty((B, C, H, W), np.float32)
    for b in range(B):
        hq = np.asarray(results[b]["hq"])    # [P, MH, H, W] bf16
        parts = results[b]["part"]           # [P, MH, NBLK] f32
        y = parts.sum(axis=2, dtype=np.float64).T.reshape(C) / (H * W)
        y1 = aw1 @ y
        a1 = 1.0 / (1.0 + np.exp(-w2v * y1))
        y2 = y1 * a1 + a2.T @ y1
        y3 = np.maximum(w3v * y2, 0.0)
        gate = 1.0 / (1.0 + np.exp(-(aw4 @ y3)))       # [C]
        hf = _bf16_to_f32(hq[:, :, :, 2 : 2 + W].view(np.uint16))
        hf = hf.transpose(1, 0, 2, 3).reshape(C, H, W)
        out[b] = hf * gate[:, None, None].astype(np.float32)
    return out


# revision 14
# speedup vs baseline: 1.0576x; 1.0576x over previous
"""Trainium2 Bass kernel for nn_DSRB_19447611916345 (dense_cnn).

Math (per batch image, C=256, H=W=128):
    S    = 0.25*(conv1x1_s1(x)+...+conv1x1_s4(x))  four (+-2,+-2)-shifted 1x1 convs
    res  = tanh(0.5*(x - S))            (= 2*sigmoid(x-S)-1)
    h    = relu(x * res)
    gate = AGCA(mean_{H,W}(h))          tiny channel-attention MLP
    out  = h * gate

Sharding: data-parallel over batch B=8 across 8 cores, weights replicated.

Device kernel (per core, one image):
  - conv matmuls in fp8e4 DoubleRow perf mode: each matmul contracts
    K=256 (both 128-channel halves paired per partition) at 0.5 cyc/row;
    weights carry a -0.25*SWC factor (sign folded so PSUM = 2^13*(x-S)).
  - a 5th bf16 matmul per tile adds +8192*x via an identity lhsT, so the
    (x - S) subtract happens inside PSUM accumulation (PE, not DVE).
  - ACT reads PSUM directly: res = tanh(2^-14*psum - 0.5*bsum) -> bf16
  - DVE: hp = x*res (bf16), then h = relu(hp) with accum_out partial sums
  - h streams to HBM as bf16 during the main loop; no phase-2 tail.
  - AGCA gate + broadcast multiply run on host in f64/f32 (tiny + exact).
  - schedule: 6-tile deep prefetch, stores lagged one tile behind compute
    (avoids SP-sequencer head-of-line blocking), trailing stores split into
    4-row chunks; DMA runs gap-free at its transfer floor.
Host prep: fp8/bf16 casts, padded fp8 copy of x, weight transpose+scale.
"""

import numpy as np
import ml_dtypes

import concourse.bacc as bacc
import concourse.mybir as mybir
import concourse.tile as tile

f32 = mybir.dt.float32
bf16 = mybir.dt.bfloat16
f8 = mybir.dt.float8e4
Alu = mybir.AluOpType
Act = mybir.ActivationFunctionType
DR = mybir.MatmulPerfMode.DoubleRow

B = 8
C = 256
H = 128
W = 128
P = 128            # SBUF partitions
KH = C // P        # 2 input-channel halves (DoubleRow pair dim)
MH = C // P        # 2 output-channel halves
RB = 4             # output rows per PSUM tile
NBLK = H // RB     # 32
NT = RB * W        # 512 = PSUM bank
PADW = W + 4       # 132
PADH = H + 4       # 132
NXQ = PADH // 8 + 1   # 17 fp8 row-tiles (last has 4 rows)
SHIFTS = [(0, 0), (4, 0), (0, 4), (4, 4)]
STORE_LAG = 2      # store tile ii at block 2*(ii+STORE_LAG)+1
SX = 32.0          # fp8 scale on x
SWC = 2048.0       # weight scale; x unscaled in fp8 (exponent folding)
PSC = 2048.0       # PSUM carries 2^11*(x-S)

NP8 = ml_dtypes.float8_e4m3
NBF = ml_dtypes.bfloat16

_STATE = {}


def _build():
    nc = bacc.Bacc(name="dsrb2")
    xb_d = nc.dram_tensor("xb", [P, MH, PADH, PADW], bf16, kind="ExternalInput")
    wl_d = nc.dram_tensor("wl", [P, len(SHIFTS), MH, KH, P], f8,
                          kind="ExternalInput")
    idn_d = nc.dram_tensor("idn", [P, P], bf16, kind="ExternalInput")
    bneg_d = nc.dram_tensor("bneg", [P, MH], f32, kind="ExternalInput")
    hq_d = nc.dram_tensor("hq", [P, MH, H, PADW], bf16, kind="ExternalOutput")
    part_d = nc.dram_tensor("part", [P, MH, NBLK], f32, kind="ExternalOutput")

    with tile.TileContext(nc) as tc:
        with (
            tc.tile_pool(name="const", bufs=1) as constp,
            tc.tile_pool(name="xq", bufs=8) as xqp,
            tc.tile_pool(name="xb", bufs=8) as xbp,
            tc.tile_pool(name="hst", bufs=5) as hstp,
            tc.tile_pool(name="mm", bufs=4) as mmp,
            tc.tile_pool(name="ps", bufs=4, space="PSUM") as psp,
        ):
            xqts, xbts, hsts = {}, {}, {}

            ncast = [0]

            def load_xb(gg):
                """Load an 8-padded-row bf16 tile; cast it to the fp8 conv
                tile on Pool (2 of 3) or DVE (1 of 3) - both have slack."""
                rows = min(8, PADH - 8 * gg)
                t = xbp.tile([P, MH, rows, PADW], bf16, tag="xb")
                nc.sync.dma_start(out=t, in_=xb_d[:, :, 8 * gg : 8 * gg + rows, :])
                xbts[gg] = t
                q = xqp.tile([P, KH, rows, PADW], f8, tag="xq")
                ceng = nc.gpsimd if ncast[0] % 3 != 2 else nc.vector
                ncast[0] += 1
                ceng.tensor_copy(out=q, in_=t)
                xqts[gg] = q

            load_xb(0)
            wt = constp.tile([P, len(SHIFTS), MH, KH, P], f8)
            nc.sync.dma_start(out=wt, in_=wl_d[:])
            idn = constp.tile([P, P], bf16)
            nc.sync.dma_start(out=idn, in_=idn_d[:, :])
            bneg = constp.tile([P, MH], f32)
            nc.sync.dma_start(out=bneg, in_=bneg_d[:, :])
            partials = constp.tile([P, MH, NBLK], f32)
            for gg in range(1, 8):
                load_xb(gg)
            for _ in range(4):
                tz = tpp.tile([P, RB, PADW], bf16, tag="t")
                nc.scalar.memzero(tz)

            for j in range(NBLK):
                ii, ro = divmod(j, 2)
                ro *= RB
                if j % 2 == 0:
                    hst_t = hstp.tile([P, MH, 8, PADW], bf16, tag="hst")
                    hsts[ii] = hst_t
                    if ii + 7 < NXQ:
                        load_xb(ii + 7)
                for mh in range(MH):
                    ps = psp.tile([P, NT], f32)
                    for s, (dr, dw) in enumerate(SHIFTS):
                        gg, off = divmod(RB * j + dr, 8)
                        rhs = xqts[gg][:, :, off : off + RB, dw : dw + W]
                        nc.tensor.matmul(
                            ps, wt[:, s, mh, :, :], rhs,
                            start=(s == 0), stop=False, perf_mode=DR,
                        )
                    g0, off0 = divmod(RB * j + 2, 8)
                    n0 = min(RB, 8 - off0)
                    nc.tensor.matmul(
                        ps[:, 0 : n0 * W], idn,
                        xbts[g0][:, mh, off0 : off0 + n0, 2 : 2 + W],
                        start=False, stop=(n0 == RB),
                    )
                    if n0 < RB:
                        nc.tensor.matmul(
                            ps[:, n0 * W : NT], idn,
                            xbts[g0 + 1][:, mh, 0 : RB - n0, 2 : 2 + W],
                            start=False, stop=True,
                        )
                    t_t = tpp.tile([P, RB, PADW], bf16, tag="t")
                    nc.scalar.activation(
                        out=t_t[:, :, 2 : 2 + W], in_=ps, func=Act.Tanh,
                        bias=bneg[:, mh : mh + 1], scale=0.5 / PSC,
                    )
                    ttw = t_t.rearrange("p a b -> p (a b)")
                    hp_t = mmp.tile([P, RB * PADW], bf16, tag="hp")
                    nc.vector.tensor_tensor(
                        out=hp_t[:, 0 : n0 * PADW],
                        in0=xbts[g0][:, mh, off0 : off0 + n0, :],
                        in1=ttw[:, 0 : n0 * PADW], op=Alu.mult,
                    )
                    if n0 < RB:
                        nc.vector.tensor_tensor(
                            out=hp_t[:, n0 * PADW :],
                            in0=xbts[g0 + 1][:, mh, 0 : RB - n0, :],
                            in1=ttw[:, n0 * PADW :], op=Alu.mult,
                        )
                    hrow = hsts[ii][:, mh, ro : ro + RB, :]
                    nc.vector.tensor_scalar(
                        out=hrow.rearrange("p a b -> p (a b)"), in0=hp_t,
                        scalar1=0.0, scalar2=0.0,
                        op0=Alu.max, op1=Alu.add,
                        accum_out=partials[:, mh, j : j + 1],
                    )
                if j % 2 == 1:
                    si = ii - STORE_LAG
                    if si >= 0:
                        for half in range(2):
                            nc.sync.dma_start(
                                out=hq_d[:, :, 8 * si + 4 * half : 8 * si + 4 * half + 4, :],
                                in_=hsts[si][:, :, 4 * half : 4 * half + 4, :],
                            )
                        hsts.pop(si, None)
            for si in range(H // 8 - STORE_LAG, H // 8):
                for half in range(2):
                    nc.sync.dma_start(
                        out=hq_d[:, :, 8 * si + 4 * half : 8 * si + 4 * half + 4, :],
                        in_=hsts[si][:, :, 4 * half : 4 * half + 4, :],
                    )
                hsts.pop(si, None)
            nc.sync.dma_start(out=part_d[:], in_=partials)

    nc.finalize()
    return nc


def _prep_core_inputs(x_img, shared):
    """x_img: [C, H, W] f32 for one batch image."""
    x4 = x_img.reshape(KH, P, H, W).transpose(1, 0, 2, 3)  # [P, KH, H, W]
    xb = np.zeros((P, MH, PADH, PADW), NBF)
    xb[:, :, 2 : H + 2, 2 : W + 2] = x4.astype(NBF)
    return {"xb": xb, **shared}


def _prep_shared(inputs):
    ws = np.stack([np.asarray(inputs[f"w{i}"]) for i in (1, 2, 3, 4)]).astype(
        np.float64
    )
    # wl[p, s, mh, kh, m] = -0.25 * SWC * w_s[mh*P+m, kh*P+p]
    wl = (-0.25 * SWC * ws).reshape(
        len(SHIFTS), MH, P, KH, P
    ).transpose(4, 0, 1, 3, 2)
    wl = np.ascontiguousarray(wl).astype(NP8)
    idn = (PSC * np.eye(P)).astype(NBF)
    bsum = 0.25 * sum(np.asarray(inputs[f"b{i}"], np.float64) for i in (1, 2, 3, 4))
    bneg = np.ascontiguousarray((-0.5 * bsum).reshape(MH, P).T).astype(np.float32)
    return {"wl": wl, "idn": idn, "bneg": bneg}


def _get_runner(nc):
    """Cached shard_map-jitted executor mirroring bass2jax.run_bass_via_pjrt's
    multi-core path, so repeat kernel() calls don't re-trace/re-jit."""
    import jax
    import concourse.mybir as mb
    from concourse import bass2jax
    from jax.sharding import Mesh, PartitionSpec
    from jax.experimental.shard_map import shard_map

    bass2jax.install_neuronx_cc_hook()
    partition_name = (
        nc.partition_id_tensor.name if nc.partition_id_tensor else None
    )
    in_names, out_names, out_avals, zero_shapes = [], [], [], []
    for alloc in nc.m.functions[0].allocations:
        if not isinstance(alloc, mb.MemoryLocationSet):
            continue
        name = alloc.memorylocations[0].name
        if alloc.kind == "ExternalInput":
            if name != partition_name:
                in_names.append(name)
        elif alloc.kind == "ExternalOutput":
            out_names.append(name)
            shape = tuple(alloc.tensor_shape)
            dtype = mb.dt.np(alloc.dtype)
            out_avals.append(jax.core.ShapedArray(shape, dtype))
            zero_shapes.append((shape, dtype))
    n_params = len(in_names)
    n_outs = len(out_avals)
    all_in_names = list(in_names) + list(out_names)
    if partition_name is not None:
        all_in_names.append(partition_name)
    donate = tuple(range(n_params, n_params + n_outs))

    def _body(*args):
        operands = list(args)
        if partition_name is not None:
            operands.append(bass2jax.partition_id_tensor())
        outs = bass2jax._bass_exec_p.bind(
            *operands,
            out_avals=tuple(out_avals),
            in_names=tuple(all_in_names),
            out_names=tuple(out_names),
            lowering_input_output_aliases=(),
            sim_require_finite=True,
            sim_require_nnan=True,
            nc=nc,
        )
        return tuple(outs)

    devices = jax.devices()[:B]
    mesh = Mesh(np.asarray(devices), ("core",))
    in_specs = (PartitionSpec("core"),) * (n_params + n_outs)
    out_specs = (PartitionSpec("core"),) * n_outs
    sharded = jax.jit(
        shard_map(_body, mesh=mesh, in_specs=in_specs, out_specs=out_specs,
                  check_rep=False),
        donate_argnums=donate,
        keep_unused=True,
    )

    def run(in_maps):
        concat_in = [
            np.concatenate([np.asarray(in_maps[c][nm]) for c in range(B)], axis=0)
            for nm in in_names
        ]
        concat_zeros = [
            np.zeros((B * s[0], *s[1:]), d) for s, d in zero_shapes
        ]
        out_arrs = sharded(*concat_in, *concat_zeros)
        return [
            {
                nm: np.asarray(out_arrs[i]).reshape(B, *out_avals[i].shape)[c]
                for i, nm in enumerate(out_names)
            }
            for c in range(B)
        ]

    return run


def _bf16_to_f32(a_u16):
    return (a_u16.astype(np.uint32) << 16).view(np.float32)


def kernel(**inputs):
    if "nc" not in _STATE:
        _STATE["nc"] = _build()
    nc = _STATE["nc"]
    x = np.asarray(inputs["x"], np.float32)
    shared = _prep_shared(inputs)
    in_maps = [_prep_core_inputs(x[b], shared) for b in range(B)]
    if "runner" not in _STATE:
        _STATE["runner"] = _get_runner(nc)
    results = _STATE["runner"](in_maps)

    # ---- host: AGCA gate (f64, tiny) + broadcast gate multiply ----
    aw1 = np.asarray(inputs["agca_w1"], np.float64)
    w2v = float(np.asarray(inputs["agca_w2"])[0])
    w3v = float(np.asarray(inputs["agca_w3"])[0])
    a2 = np.asarray(inputs["agca_A2"], np.float64)
    aw4 = np.asarray(inputs["agca_w4"], np.float64)

    out = np.empty((B, C, H, W), np.float32)
    for b in range(B):
        hq = np.asarray(results[b]["hq"])    # [P, MH, H, W] bf16
        parts = results[b]["part"]           # [P, MH, NBLK] f32
        y = parts.sum(axis=2, dtype=np.float64).T.reshape(C) / (H * W)
        y1 = aw1 @ y
        a1 = 1.0 / (1.0 + np.exp(-w2v * y1))
        y2 = y1 * a1 + a2.T @ y1
        y3 = np.maximum(w3v * y2, 0.0)
        gate = 1.0 / (1.0 + np.exp(-(aw4 @ y3)))       # [C]
        hf = _bf16_to_f32(hq[:, :, :, 2 : 2 + W].view(np.uint16))
        hf = hf.transpose(1, 0, 2, 3).reshape(C, H, W)
        out[b] = hf * gate[:, None, None].astype(np.float32)
    return out


# revision 15
# speedup vs baseline: 1.0650x; 1.0071x over previous
"""Trainium2 Bass kernel for nn_DSRB_19447611916345 (dense_cnn).

Math (per batch image, C=256, H=W=128):
    S    = 0.25*(conv1x1_s1(x)+...+conv1x1_s4(x))  four (+-2,+-2)-shifted 1x1 convs
    res  = tanh(0.5*(x - S))            (= 2*sigmoid(x-S)-1)
    h    = relu(x * res)
    gate = AGCA(mean_{H,W}(h))          tiny channel-attention MLP
    out  = h * gate

Sharding: data-parallel over batch B=8 across 8 cores, weights replicated.

Device kernel (per core, one image):
  - conv matmuls in fp8e4 DoubleRow perf mode: each matmul contracts
    K=256 (both 128-channel halves paired per partition) at 0.5 cyc/row;
    weights carry a -0.25*SWC factor (sign folded so PSUM = 2^13*(x-S)).
  - a 5th bf16 matmul per tile adds +8192*x via an identity lhsT, so the
    (x - S) subtract happens inside PSUM accumulation (PE, not DVE).
  - ACT reads PSUM directly: res = tanh(2^-14*psum - 0.5*bsum) -> bf16
  - DVE: hp = x*res (bf16), then h = relu(hp) with accum_out partial sums
  - h streams to HBM as bf16 during the main loop; no phase-2 tail.
  - AGCA gate + broadcast multiply run on host in f64/f32 (tiny + exact).
  - schedule: 6-tile deep prefetch, stores lagged one tile behind compute
    (avoids SP-sequencer head-of-line blocking), trailing stores split into
    4-row chunks; DMA runs gap-free at its transfer floor.
Host prep: fp8/bf16 casts, padded fp8 copy of x, weight transpose+scale.
"""

import numpy as np
import ml_dtypes

import concourse.bacc as bacc
import concourse.mybir as mybir
import concourse.tile as tile

f32 = mybir.dt.float32
bf16 = mybir.dt.bfloat16
f8 = mybir.dt.float8e4
Alu = mybir.AluOpType
Act = mybir.ActivationFunctionType
DR = mybir.MatmulPerfMode.DoubleRow

B = 8
C = 256
H = 128
W = 128
P = 128            # SBUF partitions
KH = C // P        # 2 input-channel halves (DoubleRow pair dim)
MH = C // P        # 2 output-channel halves
RB = 4             # output rows per PSUM tile
NBLK = H // RB     # 32
NT = RB * W        # 512 = PSUM bank
PADW = W + 4       # 132
PADH = H + 4       # 132
NXQ = PADH // 8 + 1   # 17 fp8 row-tiles (last has 4 rows)
SHIFTS = [(0, 0), (4, 0), (0, 4), (4, 4)]
STORE_LAG = 0      # store tile ii at block 2*(ii+STORE_LAG)+1
SX = 32.0          # fp8 scale on x
SWC = 2048.0       # weight scale; x unscaled in fp8 (exponent folding)
PSC = 2048.0       # PSUM carries 2^11*(x-S)

NP8 = ml_dtypes.float8_e4m3
NBF = ml_dtypes.bfloat16

_STATE = {}


def _build():
    nc = bacc.Bacc(name="dsrb2")
    xb_d = nc.dram_tensor("xb", [P, MH, PADH, PADW], bf16, kind="ExternalInput")
    wl_d = nc.dram_tensor("wl", [P, len(SHIFTS), MH, KH, P], f8,
                          kind="ExternalInput")
    idn_d = nc.dram_tensor("idn", [P, P], bf16, kind="ExternalInput")
    bneg_d = nc.dram_tensor("bneg", [P, MH], f32, kind="ExternalInput")
    hq_d = nc.dram_tensor("hq", [P, MH, H, PADW], bf16, kind="ExternalOutput")
    part_d = nc.dram_tensor("part", [P, MH, NBLK], f32, kind="ExternalOutput")

    with tile.TileContext(nc) as tc:
        with (
            tc.tile_pool(name="const", bufs=1) as constp,
            tc.tile_pool(name="xq", bufs=8) as xqp,
            tc.tile_pool(name="xb", bufs=8) as xbp,
            tc.tile_pool(name="hst", bufs=5) as hstp,
            tc.tile_pool(name="mm", bufs=4) as mmp,
            tc.tile_pool(name="ps", bufs=4, space="PSUM") as psp,
        ):
            xqts, xbts, hsts = {}, {}, {}

            ncast = [0]

            def load_xb(gg):
                """Load an 8-padded-row bf16 tile; cast it to the fp8 conv
                tile on Pool (2 of 3) or DVE (1 of 3) - both have slack."""
                rows = min(8, PADH - 8 * gg)
                t = xbp.tile([P, MH, rows, PADW], bf16, tag="xb")
                nc.sync.dma_start(out=t, in_=xb_d[:, :, 8 * gg : 8 * gg + rows, :])
                xbts[gg] = t
                q = xqp.tile([P, KH, rows, PADW], f8, tag="xq")
                ceng = nc.gpsimd if ncast[0] % 3 != 2 else nc.vector
                ncast[0] += 1
                ceng.tensor_copy(out=q, in_=t)
                xqts[gg] = q

            load_xb(0)
            wt = constp.tile([P, len(SHIFTS), MH, KH, P], f8)
            nc.sync.dma_start(out=wt, in_=wl_d[:])
            idn = constp.tile([P, P], bf16)
            nc.sync.dma_start(out=idn, in_=idn_d[:, :])
            bneg = constp.tile([P, MH], f32)
            nc.sync.dma_start(out=bneg, in_=bneg_d[:, :])
            partials = constp.tile([P, MH, NBLK], f32)
            for gg in range(1, 8):
                load_xb(gg)
            for _ in range(4):
                tz = tpp.tile([P, RB, PADW], bf16, tag="t")
                nc.scalar.memzero(tz)

            for j in range(NBLK):
                ii, ro = divmod(j, 2)
                ro *= RB
                if j % 2 == 0:
                    hst_t = hstp.tile([P, MH, 8, PADW], bf16, tag="hst")
                    hsts[ii] = hst_t
                    if ii + 7 < NXQ:
                        load_xb(ii + 7)
                for mh in range(MH):
                    ps = psp.tile([P, NT], f32)
                    for s, (dr, dw) in enumerate(SHIFTS):
                        gg, off = divmod(RB * j + dr, 8)
                        rhs = xqts[gg][:, :, off : off + RB, dw : dw + W]
                        nc.tensor.matmul(
                            ps, wt[:, s, mh, :, :], rhs,
                            start=(s == 0), stop=False, perf_mode=DR,
                        )
                    g0, off0 = divmod(RB * j + 2, 8)
                    n0 = min(RB, 8 - off0)
                    nc.tensor.matmul(
                        ps[:, 0 : n0 * W], idn,
                        xbts[g0][:, mh, off0 : off0 + n0, 2 : 2 + W],
                        start=False, stop=(n0 == RB),
                    )
                    if n0 < RB:
                        nc.tensor.matmul(
                            ps[:, n0 * W : NT], idn,
                            xbts[g0 + 1][:, mh, 0 : RB - n0, 2 : 2 + W],
                            start=False, stop=True,
                        )
                    t_t = tpp.tile([P, RB, PADW], bf16, tag="t")
                    nc.scalar.activation(
                        out=t_t[:, :, 2 : 2 + W], in_=ps, func=Act.Tanh,
                        bias=bneg[:, mh : mh + 1], scale=0.5 / PSC,
                    )
                    ttw = t_t.rearrange("p a b -> p (a b)")
                    hp_t = mmp.tile([P, RB * PADW], bf16, tag="hp")
                    nc.vector.tensor_tensor(
                        out=hp_t[:, 0 : n0 * PADW],
                        in0=xbts[g0][:, mh, off0 : off0 + n0, :],
                        in1=ttw[:, 0 : n0 * PADW], op=Alu.mult,
                    )
                    if n0 < RB:
                        nc.vector.tensor_tensor(
                            out=hp_t[:, n0 * PADW :],
                            in0=xbts[g0 + 1][:, mh, 0 : RB - n0, :],
                            in1=ttw[:, n0 * PADW :], op=Alu.mult,
                        )
                    hrow = hsts[ii][:, mh, ro : ro + RB, :]
                    nc.vector.tensor_scalar(
                        out=hrow.rearrange("p a b -> p (a b)"), in0=hp_t,
                        scalar1=0.0, scalar2=0.0,
                        op0=Alu.max, op1=Alu.add,
                        accum_out=partials[:, mh, j : j + 1],
                    )
                if j % 2 == 1:
                    si = ii - STORE_LAG
                    if si >= 0:
                        for half in range(2):
                            nc.sync.dma_start(
                                out=hq_d[:, :, 8 * si + 4 * half : 8 * si + 4 * half + 4, :],
                                in_=hsts[si][:, :, 4 * half : 4 * half + 4, :],
                            )
                        hsts.pop(si, None)
            for si in range(H // 8 - STORE_LAG, H // 8):
                for half in range(2):
                    nc.sync.dma_start(
                        out=hq_d[:, :, 8 * si + 4 * half : 8 * si + 4 * half + 4, :],
                        in_=hsts[si][:, :, 4 * half : 4 * half + 4, :],
                    )
                hsts.pop(si, None)
            nc.sync.dma_start(out=part_d[:], in_=partials)

    nc.finalize()
    return nc


def _prep_core_inputs(x_img, shared):
    """x_img: [C, H, W] f32 for one batch image."""
    x4 = x_img.reshape(KH, P, H, W).transpose(1, 0, 2, 3)  # [P, KH, H, W]
    xb = np.zeros((P, MH, PADH, PADW), NBF)
    xb[:, :, 2 : H + 2, 2 : W + 2] = x4.astype(NBF)
    return {"xb": xb, **shared}


def _prep_shared(inputs):
    ws = np.stack([np.asarray(inputs[f"w{i}"]) for i in (1, 2, 3, 4)]).astype(
        np.float64
    )
    # wl[p, s, mh, kh, m] = -0.25 * SWC * w_s[mh*P+m, kh*P+p]
    wl = (-0.25 * SWC * ws).reshape(
        len(SHIFTS), MH, P, KH, P
    ).transpose(4, 0, 1, 3, 2)
    wl = np.ascontiguousarray(wl).astype(NP8)
    idn = (PSC * np.eye(P)).astype(NBF)
    bsum = 0.25 * sum(np.asarray(inputs[f"b{i}"], np.float64) for i in (1, 2, 3, 4))
    bneg = np.ascontiguousarray((-0.5 * bsum).reshape(MH, P).T).astype(np.float32)
    return {"wl": wl, "idn": idn, "bneg": bneg}


def _get_runner(nc):
    """Cached shard_map-jitted executor mirroring bass2jax.run_bass_via_pjrt's
    multi-core path, so repeat kernel() calls don't re-trace/re-jit."""
    import jax
    import concourse.mybir as mb
    from concourse import bass2jax
    from jax.sharding import Mesh, PartitionSpec
    from jax.experimental.shard_map import shard_map

    bass2jax.install_neuronx_cc_hook()
    partition_name = (
        nc.partition_id_tensor.name if nc.partition_id_tensor else None
    )
    in_names, out_names, out_avals, zero_shapes = [], [], [], []
    for alloc in nc.m.functions[0].allocations:
        if not isinstance(alloc, mb.MemoryLocationSet):
            continue
        name = alloc.memorylocations[0].name
        if alloc.kind == "ExternalInput":
            if name != partition_name:
                in_names.append(name)
        elif alloc.kind == "ExternalOutput":
            out_names.append(name)
            shape = tuple(alloc.tensor_shape)
            dtype = mb.dt.np(alloc.dtype)
            out_avals.append(jax.core.ShapedArray(shape, dtype))
            zero_shapes.append((shape, dtype))
    n_params = len(in_names)
    n_outs = len(out_avals)
    all_in_names = list(in_names) + list(out_names)
    if partition_name is not None:
        all_in_names.append(partition_name)
    donate = tuple(range(n_params, n_params + n_outs))

    def _body(*args):
        operands = list(args)
        if partition_name is not None:
            operands.append(bass2jax.partition_id_tensor())
        outs = bass2jax._bass_exec_p.bind(
            *operands,
            out_avals=tuple(out_avals),
            in_names=tuple(all_in_names),
            out_names=tuple(out_names),
            lowering_input_output_aliases=(),
            sim_require_finite=True,
            sim_require_nnan=True,
            nc=nc,
        )
        return tuple(outs)

    devices = jax.devices()[:B]
    mesh = Mesh(np.asarray(devices), ("core",))
    in_specs = (PartitionSpec("core"),) * (n_params + n_outs)
    out_specs = (PartitionSpec("core"),) * n_outs
    sharded = jax.jit(
        shard_map(_body, mesh=mesh, in_specs=in_specs, out_specs=out_specs,
                  check_rep=False),
        donate_argnums=donate,
        keep_unused=True,
    )

    def run(in_maps):
        concat_in = [
            np.concatenate([np.asarray(in_maps[c][nm]) for c in range(B)], axis=0)
            for nm in in_names
        ]
        concat_zeros = [
            np.zeros((B * s[0], *s[1:]), d) for s, d in zero_shapes
        ]
        out_arrs = sharded(*concat_in, *concat_zeros)
        return [
            {
                nm: np.asarray(out_arrs[i]).reshape(B, *out_avals[i].shape)[c]
                for i, nm in enumerate(out_names)
            }
            for c in range(B)
        ]

    return run


def _bf16_to_f32(a_u16):
    return (a_u16.astype(np.uint32) << 16).view(np.float32)


def kernel(**inputs):
    if "nc" not in _STATE:
        _STATE["nc"] = _build()
    nc = _STATE["nc"]
    x = np.asarray(inputs["x"], np.float32)
    shared = _prep_shared(inputs)
    in_maps = [_prep_core_inputs(x[b], shared) for b in range(B)]
    if "runner" not in _STATE:
        _STATE["runner"] = _get_runner(nc)
    results = _STATE["runner"](in_maps)

    # ---- host: AGCA gate (f64, tiny) + broadcast gate multiply ----
    aw1 = np.asarray(inputs["agca_w1"], np.float64)
    w2v = float(np.asarray(inputs["agca_w2"])[0])
    w3v = float(np.asarray(inputs["agca_w3"])[0])
    a2 = np.asarray(inputs["agca_A2"], np.float64)
    aw4 = np.asarray(inputs["agca_w4"], np.float64)

    out = np.empty((B, C, H, W), np.float32)
    for b in range(B):
        hq = np.asarray(results[b]["hq"])    # [P, MH, H, W] bf16
        parts = results[b]["part"]           # [P, MH, NBLK] f32
        y = parts.sum(axis=2, dtype=np.float64).T.reshape(C) / (H * W)
        y1 = aw1 @ y
        a1 = 1.0 / (1.0 + np.exp(-w2v * y1))
        y2 = y1 * a1 + a2.T @ y1
        y3 = np.maximum(w3v * y2, 0.0)
        gate = 1.0 / (1.0 + np.exp(-(aw4 @ y3)))       # [C]
        hf = _bf16_to_f32(hq[:, :, :, 2 : 2 + W].view(np.uint16))
        hf = hf.transpose(1, 0, 2, 3).reshape(C, H, W)
        out[b] = hf * gate[:, None, None].astype(np.float32)
    return out
